# revision 1
# baseline (speedup 1.0000x reference)
"""Trainium2 Bass kernel for nn_Cache_28071906246843 (retrieval_knn).

reference semantics:
    q = h_t[cache_words]                         # [C, D] gather
    dist = sqrt(sum((cache_h - q)**2, -1))       # [C]
    vals = exp(dist / 32.0)                      # [C]
    cache_p = segment_sum(vals, cache_words, V)  # [V]
    out = log_softmax(cache_p[None, :])          # [1, V]

Sharding: cache elements are sorted by word id (pure reordering) and split
into 8 contiguous shards of 16384 elements, one per NeuronCore.  Each core
(main path, build_nc_v4): per supertile of 256 sorted elements, indirect-DMA
gathers the <=128 distinct h_t rows once (f32 -> fp16 cast in the DMA),
then the TensorEngine expands them to per-element rows and subtracts in one
step (d = I^T @ ch - S^T @ W accumulated in PSUM, with the negated one-hot
S precomputed on the host from the indices); ScalarE squares+accumulates
straight from PSUM, with batched sqrt/exp at the end.  A per-element-gather
fallback (build_nc) handles the unlikely case of >128 distinct words in a
supertile.  The [V] segment-sum of the 131072 scalars and the final
log_softmax over [V] are done on the host.
"""

import sys

import numpy as np

if "/opt/trn_rl_repo" not in sys.path:
    sys.path.insert(0, "/opt/trn_rl_repo")

import concourse.bass as bass
import concourse.tile as tile
from concourse import bacc, mybir
from concourse.bass_utils import run_bass_kernel_spmd

V, D, C = 50257, 1024, 131072
NCORES = 8
CSH = C // NCORES  # 16384 elements per core
P = 128            # SBUF partitions
NT = CSH // P      # 128 tiles per core
SMOOTH = 32.0


def build_nc(nt: int = NT, v: int = V, d: int = D) -> bass.Bass:
    """Build the per-core Bass program (SPMD: same program on all cores)."""
    nc = bacc.Bacc(
        "TRN2", target_bir_lowering=False, debug=False, num_devices=NCORES
    )
    ht = nc.dram_tensor("ht", [v, d], mybir.dt.float32, kind="ExternalInput")
    ch = nc.dram_tensor("ch", [nt * P, d], mybir.dt.float32, kind="ExternalInput")
    # cw is pre-transposed on host: cw[p, t] = word id of element t*128+p
    cw = nc.dram_tensor("cw", [P, nt], mybir.dt.int32, kind="ExternalInput")
    vals = nc.dram_tensor("vals", [P, nt], mybir.dt.float32, kind="ExternalOutput")

    ch_t = ch.ap().rearrange("(t p) d -> t p d", p=P)  # [nt, 128, d]

    with tile.TileContext(nc) as tc:
        with (
            tc.tile_pool(name="io", bufs=6) as io,
            tc.tile_pool(name="scratch", bufs=2) as scratch,
            tc.tile_pool(name="persist", bufs=1) as persist,
        ):
            cw_sb = persist.tile([P, nt], mybir.dt.int32)
            nc.sync.dma_start(out=cw_sb[:], in_=cw.ap())
            vals_sb = persist.tile([P, nt], mybir.dt.float32)
            d2_all = persist.tile([P, nt], mybir.dt.float32)

            for t in range(nt):
                ch_tile = io.tile([P, d], mybir.dt.float32, tag="ch")
                nc.sync.dma_start(out=ch_tile[:], in_=ch_t[t])

                q_tile = io.tile([P, d], mybir.dt.float32, tag="q")
                nc.gpsimd.indirect_dma_start(
                    out=q_tile[:],
                    out_offset=None,
                    in_=ht.ap(),
                    in_offset=bass.IndirectOffsetOnAxis(
                        ap=cw_sb[:, t : t + 1], axis=0
                    ),
                )

                d_tile = io.tile([P, d], mybir.dt.float32, tag="d")
                nc.vector.tensor_tensor(
                    out=d_tile[:],
                    in0=ch_tile[:],
                    in1=q_tile[:],
                    op=mybir.AluOpType.subtract,
                )

                sq_tile = scratch.tile([P, d], mybir.dt.float32, tag="sq")
                nc.scalar.activation(
                    out=sq_tile[:],
                    in_=d_tile[:],
                    func=mybir.ActivationFunctionType.Square,
                    accum_out=d2_all[:, t : t + 1],
                )

            # batched tail: one table switch each instead of two per tile
            dist_all = persist.tile([P, nt], mybir.dt.float32)
            nc.scalar.activation(
                out=dist_all[:],
                in_=d2_all[:],
                func=mybir.ActivationFunctionType.Sqrt,
            )
            nc.scalar.activation(
                out=vals_sb[:],
                in_=dist_all[:],
                func=mybir.ActivationFunctionType.Exp,
                scale=1.0 / SMOOTH,
            )

            nc.sync.dma_start(out=vals.ap(), in_=vals_sb[:])
    nc.compile()
    return nc


SUP = 2            # element-tiles per supertile
NSUP = NT // SUP   # 64 supertiles per core
SUPW = SUP * P     # 256 elements per supertile


def build_nc_v4(nt: int = NT, v: int = V, d: int = D) -> bass.Bass:
    """Dedup-gather variant.

    Per supertile (256 sorted elements): gather the <=128 distinct h_t rows
    once (cast to fp16 during the DMA).  Per element-tile, the TensorEngine
    computes d = I^T @ ch - S^T @ W directly in PSUM (host supplies the
    NEGATED one-hot S and the identity), so VectorE is not used at all.
    ScalarE squares+accumulates straight from PSUM; sqrt/exp run batched at
    the end.  ch arrives as fp16 (host cast) to halve its HBM traffic."""
    nsup = nt // SUP
    nc = bacc.Bacc(
        "TRN2", target_bir_lowering=False, debug=False, num_devices=NCORES
    )
    ht = nc.dram_tensor("ht", [v, d], mybir.dt.float32, kind="ExternalInput")
    ch = nc.dram_tensor("ch", [nt * P, d], mybir.dt.float16, kind="ExternalInput")
    # widx[p, s] = p-th (padded) distinct word id of supertile s
    widx = nc.dram_tensor("widx", [P, nsup], mybir.dt.int32, kind="ExternalInput")
    # nsel[t, w, e] = -1 iff element t*128+e's word is the w-th distinct word
    # of supertile t//SUP, else 0
    nsel = nc.dram_tensor("nsel", [nt, P, P], mybir.dt.float16, kind="ExternalInput")
    ident = nc.dram_tensor("ident", [P, P], mybir.dt.float16, kind="ExternalInput")
    vals = nc.dram_tensor("vals", [P, nt], mybir.dt.float32, kind="ExternalOutput")

    ch_ap = ch.ap()      # [nt*P, d]
    nsel_ap = nsel.ap()  # [nt, P, P]

    with tile.TileContext(nc) as tc:
        with (
            tc.tile_pool(name="io", bufs=4) as io,
            tc.tile_pool(name="wpool", bufs=3) as wpool,
            tc.tile_pool(name="spool", bufs=4) as spool,
            tc.tile_pool(name="psum", bufs=4, space="PSUM") as psum,
            tc.tile_pool(name="scratch", bufs=2) as scratch,
            tc.tile_pool(name="persist", bufs=1) as persist,
        ):
            widx_sb = persist.tile([P, nsup], mybir.dt.int32)
            nc.sync.dma_start(out=widx_sb[:], in_=widx.ap())
            ident_sb = persist.tile([P, P], mybir.dt.float16)
            nc.sync.dma_start(out=ident_sb[:], in_=ident.ap())
            vals_sb = persist.tile([P, nt], mybir.dt.float32)
            d2_all = persist.tile([P, nt], mybir.dt.float32)

            for s in range(nsup):
                w_fp = wpool.tile([P, d], mybir.dt.float16, tag="wfp")
                nc.gpsimd.indirect_dma_start(
                    out=w_fp[:],
                    out_offset=None,
                    in_=ht.ap(),
                    in_offset=bass.IndirectOffsetOnAxis(
                        ap=widx_sb[:, s : s + 1], axis=0
                    ),
                )

                # one DMA per supertile for ch ([128, SUP, d]) and nsel
                # ([128, SUP, 128])
                ch_sb = io.tile([P, SUP, d], mybir.dt.float16, tag="ch")
                ch_src = bass.AP(
                    tensor=ch_ap.tensor,
                    offset=s * SUPW * d,
                    ap=[[d, P], [P * d, SUP], [1, d]],
                )
                nc.sync.dma_start(out=ch_sb[:], in_=ch_src)

                ns_sb = spool.tile([P, SUP, P], mybir.dt.float16, tag="nsel")
                ns_src = bass.AP(
                    tensor=nsel_ap.tensor,
                    offset=s * SUP * P * P,
                    ap=[[P, P], [P * P, SUP], [1, P]],
                )
                nc.sync.dma_start(out=ns_sb[:], in_=ns_src)

                # k==0: TensorE also injects ch (d = I^T@ch - S^T@W in PSUM).
                # k==1: VectorE adds ch to the negated gather instead
                # (d = ch + (-S^T@W)), offloading half the inject matmuls.
                q_psums = []
                for k in range(SUP):
                    q_psum = psum.tile([P, d], mybir.dt.float32, tag="q")
                    q_psums.append(q_psum)
                    for h in range(0, d, 512):
                        nc.tensor.matmul(
                            out=q_psum[:, h : h + 512],
                            lhsT=ns_sb[:, k, :],
                            rhs=w_fp[:, h : h + 512],
                            start=True,
                            stop=(k == 1),
                        )
                for h in range(0, d, 512):
                    nc.tensor.matmul(
                        out=q_psums[0][:, h : h + 512],
                        lhsT=ident_sb[:],
                        rhs=ch_sb[:, 0, h : h + 512],
                        start=False,
                        stop=True,
                    )
                d_sb = io.tile([P, d], mybir.dt.float32, tag="dsb")
                nc.vector.tensor_tensor(
                    out=d_sb[:],
                    in0=ch_sb[:, 1, :],
                    in1=q_psums[1][:],
                    op=mybir.AluOpType.add,
                )
                t0 = SUP * s
                sq_tile = scratch.tile([P, d], mybir.dt.float32, tag="sq")
                nc.scalar.activation(
                    out=sq_tile[:],
                    in_=q_psums[0][:],
                    func=mybir.ActivationFunctionType.Square,
                    accum_out=d2_all[:, t0 : t0 + 1],
                )
                sq_tile2 = scratch.tile([P, d], mybir.dt.float32, tag="sq2")
                nc.scalar.activation(
                    out=sq_tile2[:],
                    in_=d_sb[:],
                    func=mybir.ActivationFunctionType.Square,
                    accum_out=d2_all[:, t0 + 1 : t0 + 2],
                )

            dist_all = persist.tile([P, nt], mybir.dt.float32)
            nc.scalar.activation(
                out=dist_all[:],
                in_=d2_all[:],
                func=mybir.ActivationFunctionType.Sqrt,
            )
            nc.scalar.activation(
                out=vals_sb[:],
                in_=dist_all[:],
                func=mybir.ActivationFunctionType.Exp,
                scale=1.0 / SMOOTH,
            )
            nc.sync.dma_start(out=vals.ap(), in_=vals_sb[:])
    nc.compile()
    return nc


def prep_v4(cw_sorted):
    """Per-core supertile metadata. Returns None if any supertile has more
    than 128 distinct words (fall back to per-element gather then)."""
    widx_all, nsel_all = [], []
    neye = -np.eye(P, dtype=np.float16)
    for c in range(NCORES):
        shard = cw_sorted[c * CSH : (c + 1) * CSH]
        widx = np.empty((NSUP, P), np.int32)
        nsel = np.empty((NT, P, P), np.float16)
        for s in range(NSUP):
            seg = shard[s * SUPW : (s + 1) * SUPW]
            uw = np.unique(seg)
            if len(uw) > P:
                return None
            widx[s, : len(uw)] = uw
            widx[s, len(uw) :] = uw[-1]
            rel = np.searchsorted(uw, seg).reshape(SUP, P)
            for k in range(SUP):
                # nsel[t][w, e] = -1 iff rel[k][e] == w
                nsel[SUP * s + k] = neye[:, rel[k]]
        widx_all.append(np.ascontiguousarray(widx.T))
        nsel_all.append(nsel)
    return widx_all, nsel_all


def make_in_maps_v4(h_t, ch_sorted, widx_all, nsel_all):
    ident = np.eye(P, dtype=np.float16)
    in_maps = []
    for c in range(NCORES):
        sl = slice(c * CSH, (c + 1) * CSH)
        in_maps.append(
            {
                "ht": h_t,
                "ch": ch_sorted[sl].astype(np.float16),
                "widx": widx_all[c],
                "nsel": nsel_all[c],
                "ident": ident,
            }
        )
    return in_maps


def make_in_maps(h_t, ch_sorted, cw_sorted):
    in_maps = []
    for c in range(NCORES):
        sl = slice(c * CSH, (c + 1) * CSH)
        in_maps.append(
            {
                "ht": h_t,
                "ch": ch_sorted[sl],
                "cw": np.ascontiguousarray(cw_sorted[sl].reshape(NT, P).T),
            }
        )
    return in_maps


def finish_on_host(vals_sorted, cw_sorted):
    """segment-sum + log_softmax (tiny O(C)+O(V) work)."""
    p = np.bincount(cw_sorted, weights=vals_sorted.astype(np.float64), minlength=V)
    m = p.max()
    lse = m + np.log(np.exp(p - m).sum())
    return (p - lse).astype(np.float32)[None, :]


def _prep(h_t, cache_h, cache_words):
    h_t = np.ascontiguousarray(np.asarray(h_t), dtype=np.float32)
    cache_h = np.ascontiguousarray(np.asarray(cache_h), dtype=np.float32)
    cw = np.asarray(cache_words).astype(np.int32)
    order = np.argsort(cw, kind="stable")
    return h_t, cache_h[order], cw[order]


def run_device(h_t, ch_sorted, cw_sorted, force_v1=False, verbose=False):
    """Compile + run the SPMD program; returns per-element vals (sorted order)."""
    import time as _time

    _t0 = _time.time()
    v4 = None if force_v1 else prep_v4(cw_sorted)
    if v4 is not None:
        nc = build_nc_v4()
        in_maps = make_in_maps_v4(h_t, ch_sorted, *v4)
    else:
        nc = build_nc()
        in_maps = make_in_maps(h_t, ch_sorted, cw_sorted)
    if verbose:
        print(f"[run_device] build+prep: {_time.time() - _t0:.1f}s")
    _t1 = _time.time()
    res = run_bass_kernel_spmd(nc, in_maps, core_ids=list(range(NCORES)))
    if verbose:
        print(f"[run_device] compile+exec: {_time.time() - _t1:.1f}s")
    return np.concatenate([r["vals"].T.reshape(-1) for r in res.results])


def kernel(h_t, cache_h, cache_words):
    h_t, ch_sorted, cw_sorted = _prep(h_t, cache_h, cache_words)
    vals_sorted = run_device(h_t, ch_sorted, cw_sorted)
    return finish_on_host(vals_sorted, cw_sorted)



# revision 2
# speedup vs baseline: 1.4545x; 1.4545x over previous
"""Trainium2 Bass kernel for nn_Cache_28071906246843 (retrieval_knn).

reference semantics:
    q = h_t[cache_words]                         # [C, D] gather
    dist = sqrt(sum((cache_h - q)**2, -1))       # [C]
    vals = exp(dist / 32.0)                      # [C]
    cache_p = segment_sum(vals, cache_words, V)  # [V]
    out = log_softmax(cache_p[None, :])          # [1, V]

v5 design (all-pairs fp8 matmul, device returns only the cross term):
    dist^2_i = ||ch_i||^2 + ||w_{r(i)}||^2 - 2 ch_i . w_{r(i)}
Both norms are host-precomputed; the device computes ONLY the selected
-2*ch.w dot per element.  Cache elements are sorted by word id and split
into 8 shards of 16384; per supertile of 256 sorted elements the <=128
distinct h_t rows (scaled by -2, cast to fp8e4m3 on host along with ch,
both pre-transposed to contraction-major [128, 8, N] blocks) meet in 16
fp8 matmuls producing the all-pairs [256 elem, 128 word] dot in PSUM.
A host-built one-hot mask picks each element's own word: a tensor_tensor
multiply plus an X-axis reduce on DVE yield [128, 2] selected dots per
supertile.  No ScalarE activations, no indirect DMAs; sqrt/exp/
segment-sum/log_softmax run on the host.  The v4 dedup-gather kernel is
kept as a fallback in case a supertile exceeds 128 distinct words.
"""

import sys

import numpy as np

if "/opt/trn_rl_repo" not in sys.path:
    sys.path.insert(0, "/opt/trn_rl_repo")

import ml_dtypes

import concourse.bass as bass
import concourse.tile as tile
from concourse import bacc, mybir
from concourse.bass_utils import run_bass_kernel_spmd

V, D, C = 50257, 1024, 131072
NCORES = 8
CSH = C // NCORES  # 16384 elements per core
P = 128            # SBUF partitions
NT = CSH // P      # 128 element-tiles per core
SMOOTH = 32.0

SUP = 2            # element-tiles per supertile
NSUP = NT // SUP   # 64 supertiles per core
SUPW = SUP * P     # 256 elements per supertile
NCH = D // P       # 8 contraction chunks

FP8 = ml_dtypes.float8_e4m3


def build_nc_v5(nsup: int = NSUP) -> bass.Bass:
    """All-pairs dot kernel.  Per supertile s:
      chb[s]: [128, 8, 256] fp8, chb[s][p][c][u] = ch[s*256+u, c*128+p]
      wb[s]:  [128, 8, 128] fp8, wb[s][p][c][j] = -2*ht[widx[s][j], c*128+p]
      ohb[s]: [128, 2, 128] fp8, ohb[s][m][g][w] = 1 iff elem g*128+m selects w
    PE: psum[m, g*128+w] = sum_c sum_p chb.T @ wb  (elements stationary, FWL)
    DVE: tmp = psum * ohb ; dsel[:, 2s+g] = reduce_X(tmp)
    """
    nc = bacc.Bacc(
        "TRN2", target_bir_lowering=False, debug=False, num_devices=NCORES
    )
    chb = nc.dram_tensor(
        "chb", [nsup, P, NCH * SUPW], mybir.dt.float8e4, kind="ExternalInput"
    )
    wb = nc.dram_tensor(
        "wb", [nsup, P, NCH * P], mybir.dt.float8e4, kind="ExternalInput"
    )
    ohb = nc.dram_tensor(
        "ohb", [nsup, P, SUP * P], mybir.dt.float8e4, kind="ExternalInput"
    )
    dsel = nc.dram_tensor(
        "dsel", [P, SUP * nsup], mybir.dt.float32, kind="ExternalOutput"
    )

    chb_ap = chb.ap()  # [nsup, 128, 2048]
    wb_ap = wb.ap()    # [nsup, 128, 1024]
    ohb_ap = ohb.ap()  # [nsup, 128, 256]

    # split the chb load across both HWDGE rings to balance HBM traffic:
    # sync carries chunks [0:CSPL), scalar carries [CSPL:8) + wb + ohb.
    CSPL = 6

    with tile.TileContext(nc) as tc:
        with (
            tc.tile_pool(name="io", bufs=4) as io,
            tc.tile_pool(name="ohp", bufs=4) as ohp,
            tc.tile_pool(name="tmpp", bufs=3) as tmpp,
            tc.tile_pool(name="psum", bufs=4, space="PSUM") as psum,
            tc.tile_pool(name="persist", bufs=1) as persist,
        ):
            dsel_sb = persist.tile([P, SUP * nsup], mybir.dt.float32)

            for s in range(nsup):
                ch_sb = io.tile([P, NCH, SUPW], mybir.dt.float8e4, tag="ch")
                nc.sync.dma_start(
                    out=ch_sb[:, 0:CSPL, :],
                    in_=chb_ap[s][:, 0 : CSPL * SUPW],
                )
                nc.scalar.dma_start(
                    out=ch_sb[:, CSPL:NCH, :],
                    in_=chb_ap[s][:, CSPL * SUPW : NCH * SUPW],
                )
                w_sb = io.tile([P, NCH, P], mybir.dt.float8e4, tag="w")
                nc.scalar.dma_start(out=w_sb[:], in_=wb_ap[s])
                oh_sb = ohp.tile([P, SUP, P], mybir.dt.float8e4, tag="oh")
                nc.scalar.dma_start(out=oh_sb[:], in_=ohb_ap[s])

                pt = psum.tile([P, SUP * P], mybir.dt.float32, tag="pt")
                for g in range(SUP):
                    for c in range(NCH):
                        nc.tensor.matmul(
                            out=pt[:, g * P : (g + 1) * P],
                            lhsT=ch_sb[:, c, g * P : (g + 1) * P],
                            rhs=w_sb[:, c, :],
                            start=(c == 0),
                            stop=(c == NCH - 1),
                        )

                tmp = tmpp.tile([P, SUP, P], mybir.dt.float32, tag="tmp")
                nc.vector.tensor_tensor(
                    out=tmp[:],
                    in0=pt[:],
                    in1=oh_sb[:],
                    op=mybir.AluOpType.mult,
                )
                nc.vector.tensor_reduce(
                    out=dsel_sb[:, SUP * s : SUP * (s + 1)],
                    in_=tmp[:],
                    axis=mybir.AxisListType.X,
                    op=mybir.AluOpType.add,
                )

            nc.gpsimd.dma_start(out=dsel.ap(), in_=dsel_sb[:])
    nc.compile()
    return nc


def prep_v5(h_t, ch_sorted, cw_sorted):
    """Host-side block building for v5.  Returns None if any supertile has
    more than 128 distinct words (fall back to v4 then)."""
    S = NCORES * NSUP  # 512 supertiles total
    seg = cw_sorted.reshape(S, SUPW)
    widx = np.empty((S, P), np.int64)
    rel = np.empty((S, SUPW), np.int64)
    for s in range(S):
        uw, r = np.unique(seg[s], return_inverse=True)
        if len(uw) > P:
            return None
        widx[s, : len(uw)] = uw
        widx[s, len(uw):] = uw[-1]
        rel[s] = r

    ht8 = (-2.0 * h_t).astype(FP8)
    ch8 = ch_sorted.astype(FP8)

    # chb[s, p, c, u] = ch8[s*256+u, c*128+p]
    chb = np.ascontiguousarray(
        ch8.reshape(S, SUPW, NCH, P).transpose(0, 3, 2, 1)
    ).reshape(S, P, NCH * SUPW)
    # wb[s, p, c, j] = ht8[widx[s, j], c*128+p]
    wb = np.ascontiguousarray(
        ht8[widx].reshape(S, P, NCH, P).transpose(0, 3, 2, 1)
    ).reshape(S, P, NCH * P)
    # ohb[s, m, g, w] = 1 iff rel[s, g*128+m] == w
    ohb = np.zeros((S, P, SUP, P), FP8)
    s_i = np.repeat(np.arange(S), SUPW)
    u_i = np.tile(np.arange(SUPW), S)
    g_i, m_i = u_i // P, u_i % P
    ohb[s_i, m_i, g_i, rel.reshape(-1)] = 1.0
    ohb = ohb.reshape(S, P, SUP * P)

    hn2 = np.einsum("ij,ij->i", h_t, h_t, dtype=np.float64)
    cn2 = np.einsum("ij,ij->i", ch_sorted, ch_sorted, dtype=np.float64)
    b = cn2 + hn2[cw_sorted]  # [C] norm part of dist^2, in sorted order
    return chb, wb, ohb, b


def make_in_maps_v5(chb, wb, ohb):
    in_maps = []
    for c in range(NCORES):
        sl = slice(c * NSUP, (c + 1) * NSUP)
        in_maps.append({"chb": chb[sl], "wb": wb[sl], "ohb": ohb[sl]})
    return in_maps


# ---------------------------------------------------------------------------
# v4 fallback (dedup gather + TensorE expand/subtract + ScalarE square)
# ---------------------------------------------------------------------------


def build_nc_v4(nt: int = NT, v: int = V, d: int = D) -> bass.Bass:
    nsup = nt // SUP
    nc = bacc.Bacc(
        "TRN2", target_bir_lowering=False, debug=False, num_devices=NCORES
    )
    ht = nc.dram_tensor("ht", [v, d], mybir.dt.float32, kind="ExternalInput")
    ch = nc.dram_tensor("ch", [nt * P, d], mybir.dt.float16, kind="ExternalInput")
    widx = nc.dram_tensor("widx", [P, nsup], mybir.dt.int32, kind="ExternalInput")
    nsel = nc.dram_tensor("nsel", [nt, P, P], mybir.dt.float16, kind="ExternalInput")
    ident = nc.dram_tensor("ident", [P, P], mybir.dt.float16, kind="ExternalInput")
    vals = nc.dram_tensor("vals", [P, nt], mybir.dt.float32, kind="ExternalOutput")

    ch_ap = ch.ap()
    nsel_ap = nsel.ap()

    with tile.TileContext(nc) as tc:
        with (
            tc.tile_pool(name="io", bufs=4) as io,
            tc.tile_pool(name="wpool", bufs=3) as wpool,
            tc.tile_pool(name="spool", bufs=4) as spool,
            tc.tile_pool(name="psum", bufs=4, space="PSUM") as psum,
            tc.tile_pool(name="scratch", bufs=2) as scratch,
            tc.tile_pool(name="persist", bufs=1) as persist,
        ):
            widx_sb = persist.tile([P, nsup], mybir.dt.int32)
            nc.sync.dma_start(out=widx_sb[:], in_=widx.ap())
            ident_sb = persist.tile([P, P], mybir.dt.float16)
            nc.sync.dma_start(out=ident_sb[:], in_=ident.ap())
            vals_sb = persist.tile([P, nt], mybir.dt.float32)
            d2_all = persist.tile([P, nt], mybir.dt.float32)

            for s in range(nsup):
                w_fp = wpool.tile([P, d], mybir.dt.float16, tag="wfp")
                nc.gpsimd.indirect_dma_start(
                    out=w_fp[:],
                    out_offset=None,
                    in_=ht.ap(),
                    in_offset=bass.IndirectOffsetOnAxis(
                        ap=widx_sb[:, s : s + 1], axis=0
                    ),
                )
                ch_sb = io.tile([P, SUP, d], mybir.dt.float16, tag="ch")
                ch_src = bass.AP(
                    tensor=ch_ap.tensor,
                    offset=s * SUPW * d,
                    ap=[[d, P], [P * d, SUP], [1, d]],
                )
                nc.sync.dma_start(out=ch_sb[:], in_=ch_src)
                ns_sb = spool.tile([P, SUP, P], mybir.dt.float16, tag="nsel")
                ns_src = bass.AP(
                    tensor=nsel_ap.tensor,
                    offset=s * SUP * P * P,
                    ap=[[P, P], [P * P, SUP], [1, P]],
                )
                nc.sync.dma_start(out=ns_sb[:], in_=ns_src)

                q_psums = []
                for k in range(SUP):
                    q_psum = psum.tile([P, d], mybir.dt.float32, tag="q")
                    q_psums.append(q_psum)
                    for h in range(0, d, 512):
                        nc.tensor.matmul(
                            out=q_psum[:, h : h + 512],
                            lhsT=ns_sb[:, k, :],
                            rhs=w_fp[:, h : h + 512],
                            start=True,
                            stop=(k == 1),
                        )
                for h in range(0, d, 512):
                    nc.tensor.matmul(
                        out=q_psums[0][:, h : h + 512],
                        lhsT=ident_sb[:],
                        rhs=ch_sb[:, 0, h : h + 512],
                        start=False,
                        stop=True,
                    )
                d_sb = io.tile([P, d], mybir.dt.float32, tag="dsb")
                nc.vector.tensor_tensor(
                    out=d_sb[:],
                    in0=ch_sb[:, 1, :],
                    in1=q_psums[1][:],
                    op=mybir.AluOpType.add,
                )
                t0 = SUP * s
                sq_tile = scratch.tile([P, d], mybir.dt.float32, tag="sq")
                nc.scalar.activation(
                    out=sq_tile[:],
                    in_=q_psums[0][:],
                    func=mybir.ActivationFunctionType.Square,
                    accum_out=d2_all[:, t0 : t0 + 1],
                )
                sq_tile2 = scratch.tile([P, d], mybir.dt.float32, tag="sq2")
                nc.scalar.activation(
                    out=sq_tile2[:],
                    in_=d_sb[:],
                    func=mybir.ActivationFunctionType.Square,
                    accum_out=d2_all[:, t0 + 1 : t0 + 2],
                )

            dist_all = persist.tile([P, nt], mybir.dt.float32)
            nc.scalar.activation(
                out=dist_all[:],
                in_=d2_all[:],
                func=mybir.ActivationFunctionType.Sqrt,
            )
            nc.scalar.activation(
                out=vals_sb[:],
                in_=dist_all[:],
                func=mybir.ActivationFunctionType.Exp,
                scale=1.0 / SMOOTH,
            )
            nc.sync.dma_start(out=vals.ap(), in_=vals_sb[:])
    nc.compile()
    return nc


def prep_v4(cw_sorted):
    widx_all, nsel_all = [], []
    neye = -np.eye(P, dtype=np.float16)
    for c in range(NCORES):
        shard = cw_sorted[c * CSH : (c + 1) * CSH]
        widx = np.empty((NSUP, P), np.int32)
        nsel = np.empty((NT, P, P), np.float16)
        for s in range(NSUP):
            seg = shard[s * SUPW : (s + 1) * SUPW]
            uw = np.unique(seg)
            if len(uw) > P:
                return None
            widx[s, : len(uw)] = uw
            widx[s, len(uw) :] = uw[-1]
            rel = np.searchsorted(uw, seg).reshape(SUP, P)
            for k in range(SUP):
                nsel[SUP * s + k] = neye[:, rel[k]]
        widx_all.append(np.ascontiguousarray(widx.T))
        nsel_all.append(nsel)
    return widx_all, nsel_all


def make_in_maps_v4(h_t, ch_sorted, widx_all, nsel_all):
    ident = np.eye(P, dtype=np.float16)
    in_maps = []
    for c in range(NCORES):
        sl = slice(c * CSH, (c + 1) * CSH)
        in_maps.append(
            {
                "ht": h_t,
                "ch": ch_sorted[sl].astype(np.float16),
                "widx": widx_all[c],
                "nsel": nsel_all[c],
                "ident": ident,
            }
        )
    return in_maps


def finish_on_host(vals_sorted, cw_sorted):
    """segment-sum + log_softmax (tiny O(C)+O(V) work)."""
    p = np.bincount(cw_sorted, weights=vals_sorted.astype(np.float64), minlength=V)
    m = p.max()
    lse = m + np.log(np.exp(p - m).sum())
    return (p - lse).astype(np.float32)[None, :]


def _prep(h_t, cache_h, cache_words):
    h_t = np.ascontiguousarray(np.asarray(h_t), dtype=np.float32)
    cache_h = np.ascontiguousarray(np.asarray(cache_h), dtype=np.float32)
    cw = np.asarray(cache_words).astype(np.int32)
    order = np.argsort(cw, kind="stable")
    return h_t, cache_h[order], cw[order]


def run_device(h_t, ch_sorted, cw_sorted, force_v1=False, verbose=False):
    """Compile + run the SPMD program; returns per-element vals (sorted order)."""
    import time as _time

    _t0 = _time.time()
    v5 = prep_v5(h_t, ch_sorted, cw_sorted)
    if v5 is not None:
        chb, wb, ohb, b = v5
        nc = build_nc_v5()
        in_maps = make_in_maps_v5(chb, wb, ohb)
        if verbose:
            print(f"[run_device] build+prep(v5): {_time.time() - _t0:.1f}s")
        _t1 = _time.time()
        res = run_bass_kernel_spmd(nc, in_maps, core_ids=list(range(NCORES)))
        if verbose:
            print(f"[run_device] compile+exec: {_time.time() - _t1:.1f}s")
        # dsel[p, t] = selected -2*dot for element t*128+p (per core)
        dsel = np.concatenate(
            [r["dsel"].T.reshape(-1) for r in res.results]
        ).astype(np.float64)
        d2 = np.maximum(b + dsel, 0.0)
        return np.exp(np.sqrt(d2) / SMOOTH)

    v4 = prep_v4(cw_sorted)
    assert v4 is not None, "both v5 and v4 prep failed"
    nc = build_nc_v4()
    in_maps = make_in_maps_v4(h_t, ch_sorted, *v4)
    if verbose:
        print(f"[run_device] build+prep(v4): {_time.time() - _t0:.1f}s")
    _t1 = _time.time()
    res = run_bass_kernel_spmd(nc, in_maps, core_ids=list(range(NCORES)))
    if verbose:
        print(f"[run_device] compile+exec: {_time.time() - _t1:.1f}s")
    return np.concatenate([r["vals"].T.reshape(-1) for r in res.results])


def kernel(h_t, cache_h, cache_words):
    h_t, ch_sorted, cw_sorted = _prep(h_t, cache_h, cache_words)
    vals_sorted = run_device(h_t, ch_sorted, cw_sorted)
    return finish_on_host(vals_sorted, cw_sorted)


# revision 6
# speedup vs baseline: 2.2590x; 1.5531x over previous
"""Trainium2 Bass kernel for nn_Cache_28071906246843 (retrieval_knn).

reference semantics:
    q = h_t[cache_words]                         # [C, D] gather
    dist = sqrt(sum((cache_h - q)**2, -1))       # [C]
    vals = exp(dist / 32.0)                      # [C]
    cache_p = segment_sum(vals, cache_words, V)  # [V]
    out = log_softmax(cache_p[None, :])          # [1, V]

v5 design (all-pairs fp8 matmul, device returns only the cross term):
    dist^2_i = ||ch_i||^2 + ||w_{r(i)}||^2 - 2 ch_i . w_{r(i)}
Both norms are host-precomputed; the device computes ONLY the selected
-2*ch.w dot per element.  Cache elements are sorted by word id and split
into 8 shards of 16384; per supertile of 256 sorted elements the <=128
distinct h_t rows (scaled by -2, cast to fp8e4m3 on host along with ch,
both pre-transposed to contraction-major [128, 8, N] blocks) meet in 16
fp8 matmuls producing the all-pairs [256 elem, 128 word] dot in PSUM.
A host-built one-hot mask picks each element's own word: a tensor_tensor
multiply plus an X-axis reduce on DVE yield [128, 2] selected dots per
supertile.  No ScalarE activations, no indirect DMAs; sqrt/exp/
segment-sum/log_softmax run on the host.  The v4 dedup-gather kernel is
kept as a fallback in case a supertile exceeds 128 distinct words.
"""

import sys

import numpy as np

if "/opt/trn_rl_repo" not in sys.path:
    sys.path.insert(0, "/opt/trn_rl_repo")

import ml_dtypes

import concourse.bass as bass
import concourse.tile as tile
from concourse import bacc, mybir
from concourse.bass_utils import run_bass_kernel_spmd

V, D, C = 50257, 1024, 131072
NCORES = 8
CSH = C // NCORES  # 16384 elements per core
P = 128            # SBUF partitions
NT = CSH // P      # 128 element-tiles per core
SMOOTH = 32.0

SUP = 2            # element-tiles per supertile
NSUP = NT // SUP   # 64 supertiles per core
SUPW = SUP * P     # 256 elements per supertile
NCH = D // P       # 8 contraction chunks

FP8 = ml_dtypes.float8_e4m3


def build_nc_v5(nsup: int = NSUP) -> bass.Bass:
    """All-pairs dot kernel.  Per supertile s:
      chb[s]: [128, 8, 256] fp8, chb[s][p][c][u] = ch[s*256+u, c*128+p]
      wb[s]:  [128, 8, 128] fp8, wb[s][p][c][j] = -2*ht[widx[s][j], c*128+p]
      ohb[s]: [128, 2, 128] fp8, ohb[s][m][g][w] = 1 iff elem g*128+m selects w
    PE: psum[m, g*128+w] = sum_c sum_p chb.T @ wb  (elements stationary, FWL)
    DVE: tmp = psum * ohb ; dsel[:, 2s+g] = reduce_X(tmp)
    """
    nc = bacc.Bacc(
        "TRN2", target_bir_lowering=False, debug=False, num_devices=NCORES
    )
    chb = nc.dram_tensor(
        "chb", [nsup, P, NCH * SUPW], mybir.dt.float8e4, kind="ExternalInput"
    )
    # wob = per-sup word block (8 chunks of -2*ht rows) ++ one-hot (2 tiles)
    wob = nc.dram_tensor(
        "wob", [nsup, P, (NCH + SUP) * P], mybir.dt.float8e4, kind="ExternalInput"
    )
    dsel = nc.dram_tensor(
        "dsel", [P, SUP * nsup], mybir.dt.float32, kind="ExternalOutput"
    )

    chb_ap = chb.ap()  # [nsup, 128, 2048]
    wob_ap = wob.ap()  # [nsup, 128, 1280]

    with tile.TileContext(nc) as tc:
        with (
            tc.tile_pool(name="io", bufs=6) as io,
            tc.tile_pool(name="tmpp", bufs=4) as tmpp,
            tc.tile_pool(name="psum", bufs=6, space="PSUM") as psum,
            tc.tile_pool(name="persist", bufs=1) as persist,
        ):
            dsel_sb = persist.tile([P, SUP * nsup], mybir.dt.float32)

            for s in range(nsup):
                ch_sb = io.tile([P, NCH, SUPW], mybir.dt.float8e4, tag="ch")
                nc.sync.dma_start(out=ch_sb[:], in_=chb_ap[s])
                w_sb = io.tile([P, NCH + SUP, P], mybir.dt.float8e4, tag="w")
                nc.scalar.dma_start(out=w_sb[:], in_=wob_ap[s])

                pt = psum.tile([P, SUP * P], mybir.dt.float32, tag="pt")
                for g in range(SUP):
                    for c in range(NCH):
                        nc.tensor.matmul(
                            out=pt[:, g * P : (g + 1) * P],
                            lhsT=ch_sb[:, c, g * P : (g + 1) * P],
                            rhs=w_sb[:, c, :],
                            start=(c == 0),
                            stop=(c == NCH - 1),
                        )

                tmp = tmpp.tile([P, SUP, P], mybir.dt.float32, tag="tmp")
                nc.vector.tensor_tensor(
                    out=tmp[:],
                    in0=pt[:],
                    in1=w_sb[:, NCH : NCH + SUP, :],
                    op=mybir.AluOpType.mult,
                )
                nc.vector.tensor_reduce(
                    out=dsel_sb[:, SUP * s : SUP * (s + 1)],
                    in_=tmp[:],
                    axis=mybir.AxisListType.X,
                    op=mybir.AluOpType.add,
                )

            nc.gpsimd.dma_start(out=dsel.ap(), in_=dsel_sb[:])
    nc.compile()
    return nc


def prep_v5(h_t, ch_sorted, cw_sorted):
    """Host-side block building for v5.  Returns None if any supertile has
    more than 128 distinct words (fall back to v4 then)."""
    S = NCORES * NSUP  # 512 supertiles total
    seg = cw_sorted.reshape(S, SUPW)
    widx = np.empty((S, P), np.int64)
    rel = np.empty((S, SUPW), np.int64)
    for s in range(S):
        uw, r = np.unique(seg[s], return_inverse=True)
        if len(uw) > P:
            return None
        widx[s, : len(uw)] = uw
        widx[s, len(uw):] = uw[-1]
        rel[s] = r

    ht8 = (-2.0 * h_t).astype(FP8)
    ch8 = ch_sorted.astype(FP8)

    # chb[s, p, c, u] = ch8[s*256+u, c*128+p]
    chb = np.ascontiguousarray(
        ch8.reshape(S, SUPW, NCH, P).transpose(0, 3, 2, 1)
    ).reshape(S, P, NCH * SUPW)
    # wb[s, p, c, j] = ht8[widx[s, j], c*128+p]
    wb = np.ascontiguousarray(
        ht8[widx].reshape(S, P, NCH, P).transpose(0, 3, 2, 1)
    ).reshape(S, P, NCH * P)
    # ohb[s, m, g, w] = 1 iff rel[s, g*128+m] == w
    ohb = np.zeros((S, P, SUP, P), FP8)
    s_i = np.repeat(np.arange(S), SUPW)
    u_i = np.tile(np.arange(SUPW), S)
    g_i, m_i = u_i // P, u_i % P
    ohb[s_i, m_i, g_i, rel.reshape(-1)] = 1.0
    ohb = ohb.reshape(S, P, SUP * P)

    wob = np.ascontiguousarray(
        np.concatenate([wb, ohb], axis=-1)
    )  # [S, P, 1280]

    hn2 = np.einsum("ij,ij->i", h_t, h_t, dtype=np.float64)
    cn2 = np.einsum("ij,ij->i", ch_sorted, ch_sorted, dtype=np.float64)
    b = cn2 + hn2[cw_sorted]  # [C] norm part of dist^2, in sorted order
    return chb, wob, b


def make_in_maps_v5(chb, wob):
    in_maps = []
    for c in range(NCORES):
        sl = slice(c * NSUP, (c + 1) * NSUP)
        in_maps.append({"chb": chb[sl], "wob": wob[sl]})
    return in_maps


# ---------------------------------------------------------------------------
# v4 fallback (dedup gather + TensorE expand/subtract + ScalarE square)
# ---------------------------------------------------------------------------


def build_nc_v4(nt: int = NT, v: int = V, d: int = D) -> bass.Bass:
    nsup = nt // SUP
    nc = bacc.Bacc(
        "TRN2", target_bir_lowering=False, debug=False, num_devices=NCORES
    )
    ht = nc.dram_tensor("ht", [v, d], mybir.dt.float32, kind="ExternalInput")
    ch = nc.dram_tensor("ch", [nt * P, d], mybir.dt.float16, kind="ExternalInput")
    widx = nc.dram_tensor("widx", [P, nsup], mybir.dt.int32, kind="ExternalInput")
    nsel = nc.dram_tensor("nsel", [nt, P, P], mybir.dt.float16, kind="ExternalInput")
    ident = nc.dram_tensor("ident", [P, P], mybir.dt.float16, kind="ExternalInput")
    vals = nc.dram_tensor("vals", [P, nt], mybir.dt.float32, kind="ExternalOutput")

    ch_ap = ch.ap()
    nsel_ap = nsel.ap()

    with tile.TileContext(nc) as tc:
        with (
            tc.tile_pool(name="io", bufs=4) as io,
            tc.tile_pool(name="wpool", bufs=3) as wpool,
            tc.tile_pool(name="spool", bufs=4) as spool,
            tc.tile_pool(name="psum", bufs=4, space="PSUM") as psum,
            tc.tile_pool(name="scratch", bufs=2) as scratch,
            tc.tile_pool(name="persist", bufs=1) as persist,
        ):
            widx_sb = persist.tile([P, nsup], mybir.dt.int32)
            nc.sync.dma_start(out=widx_sb[:], in_=widx.ap())
            ident_sb = persist.tile([P, P], mybir.dt.float16)
            nc.sync.dma_start(out=ident_sb[:], in_=ident.ap())
            vals_sb = persist.tile([P, nt], mybir.dt.float32)
            d2_all = persist.tile([P, nt], mybir.dt.float32)

            for s in range(nsup):
                w_fp = wpool.tile([P, d], mybir.dt.float16, tag="wfp")
                nc.gpsimd.indirect_dma_start(
                    out=w_fp[:],
                    out_offset=None,
                    in_=ht.ap(),
                    in_offset=bass.IndirectOffsetOnAxis(
                        ap=widx_sb[:, s : s + 1], axis=0
                    ),
                )
                ch_sb = io.tile([P, SUP, d], mybir.dt.float16, tag="ch")
                ch_src = bass.AP(
                    tensor=ch_ap.tensor,
                    offset=s * SUPW * d,
                    ap=[[d, P], [P * d, SUP], [1, d]],
                )
                nc.sync.dma_start(out=ch_sb[:], in_=ch_src)
                ns_sb = spool.tile([P, SUP, P], mybir.dt.float16, tag="nsel")
                ns_src = bass.AP(
                    tensor=nsel_ap.tensor,
                    offset=s * SUP * P * P,
                    ap=[[P, P], [P * P, SUP], [1, P]],
                )
                nc.sync.dma_start(out=ns_sb[:], in_=ns_src)

                q_psums = []
                for k in range(SUP):
                    q_psum = psum.tile([P, d], mybir.dt.float32, tag="q")
                    q_psums.append(q_psum)
                    for h in range(0, d, 512):
                        nc.tensor.matmul(
                            out=q_psum[:, h : h + 512],
                            lhsT=ns_sb[:, k, :],
                            rhs=w_fp[:, h : h + 512],
                            start=True,
                            stop=(k == 1),
                        )
                for h in range(0, d, 512):
                    nc.tensor.matmul(
                        out=q_psums[0][:, h : h + 512],
                        lhsT=ident_sb[:],
                        rhs=ch_sb[:, 0, h : h + 512],
                        start=False,
                        stop=True,
                    )
                d_sb = io.tile([P, d], mybir.dt.float32, tag="dsb")
                nc.vector.tensor_tensor(
                    out=d_sb[:],
                    in0=ch_sb[:, 1, :],
                    in1=q_psums[1][:],
                    op=mybir.AluOpType.add,
                )
                t0 = SUP * s
                sq_tile = scratch.tile([P, d], mybir.dt.float32, tag="sq")
                nc.scalar.activation(
                    out=sq_tile[:],
                    in_=q_psums[0][:],
                    func=mybir.ActivationFunctionType.Square,
                    accum_out=d2_all[:, t0 : t0 + 1],
                )
                sq_tile2 = scratch.tile([P, d], mybir.dt.float32, tag="sq2")
                nc.scalar.activation(
                    out=sq_tile2[:],
                    in_=d_sb[:],
                    func=mybir.ActivationFunctionType.Square,
                    accum_out=d2_all[:, t0 + 1 : t0 + 2],
                )

            dist_all = persist.tile([P, nt], mybir.dt.float32)
            nc.scalar.activation(
                out=dist_all[:],
                in_=d2_all[:],
                func=mybir.ActivationFunctionType.Sqrt,
            )
            nc.scalar.activation(
                out=vals_sb[:],
                in_=dist_all[:],
                func=mybir.ActivationFunctionType.Exp,
                scale=1.0 / SMOOTH,
            )
            nc.sync.dma_start(out=vals.ap(), in_=vals_sb[:])
    nc.compile()
    return nc


def prep_v4(cw_sorted):
    widx_all, nsel_all = [], []
    neye = -np.eye(P, dtype=np.float16)
    for c in range(NCORES):
        shard = cw_sorted[c * CSH : (c + 1) * CSH]
        widx = np.empty((NSUP, P), np.int32)
        nsel = np.empty((NT, P, P), np.float16)
        for s in range(NSUP):
            seg = shard[s * SUPW : (s + 1) * SUPW]
            uw = np.unique(seg)
            if len(uw) > P:
                return None
            widx[s, : len(uw)] = uw
            widx[s, len(uw) :] = uw[-1]
            rel = np.searchsorted(uw, seg).reshape(SUP, P)
            for k in range(SUP):
                nsel[SUP * s + k] = neye[:, rel[k]]
        widx_all.append(np.ascontiguousarray(widx.T))
        nsel_all.append(nsel)
    return widx_all, nsel_all


def make_in_maps_v4(h_t, ch_sorted, widx_all, nsel_all):
    ident = np.eye(P, dtype=np.float16)
    in_maps = []
    for c in range(NCORES):
        sl = slice(c * CSH, (c + 1) * CSH)
        in_maps.append(
            {
                "ht": h_t,
                "ch": ch_sorted[sl].astype(np.float16),
                "widx": widx_all[c],
                "nsel": nsel_all[c],
                "ident": ident,
            }
        )
    return in_maps


def finish_on_host(vals_sorted, cw_sorted):
    """segment-sum + log_softmax (tiny O(C)+O(V) work)."""
    p = np.bincount(cw_sorted, weights=vals_sorted.astype(np.float64), minlength=V)
    m = p.max()
    lse = m + np.log(np.exp(p - m).sum())
    return (p - lse).astype(np.float32)[None, :]


def _prep(h_t, cache_h, cache_words):
    h_t = np.ascontiguousarray(np.asarray(h_t), dtype=np.float32)
    cache_h = np.ascontiguousarray(np.asarray(cache_h), dtype=np.float32)
    cw = np.asarray(cache_words).astype(np.int32)
    order = np.argsort(cw, kind="stable")
    return h_t, cache_h[order], cw[order]


def run_device(h_t, ch_sorted, cw_sorted, force_v1=False, verbose=False):
    """Compile + run the SPMD program; returns per-element vals (sorted order)."""
    import time as _time

    _t0 = _time.time()
    v5 = prep_v5(h_t, ch_sorted, cw_sorted)
    if v5 is not None:
        chb, wob, b = v5
        nc = build_nc_v5()
        in_maps = make_in_maps_v5(chb, wob)
        if verbose:
            print(f"[run_device] build+prep(v5): {_time.time() - _t0:.1f}s")
        _t1 = _time.time()
        res = run_bass_kernel_spmd(nc, in_maps, core_ids=list(range(NCORES)))
        if verbose:
            print(f"[run_device] compile+exec: {_time.time() - _t1:.1f}s")
        # dsel[p, t] = selected -2*dot for element t*128+p (per core)
        dsel = np.concatenate(
            [r["dsel"].T.reshape(-1) for r in res.results]
        ).astype(np.float64)
        d2 = np.maximum(b + dsel, 0.0)
        return np.exp(np.sqrt(d2) / SMOOTH)

    v4 = prep_v4(cw_sorted)
    assert v4 is not None, "both v5 and v4 prep failed"
    nc = build_nc_v4()
    in_maps = make_in_maps_v4(h_t, ch_sorted, *v4)
    if verbose:
        print(f"[run_device] build+prep(v4): {_time.time() - _t0:.1f}s")
    _t1 = _time.time()
    res = run_bass_kernel_spmd(nc, in_maps, core_ids=list(range(NCORES)))
    if verbose:
        print(f"[run_device] compile+exec: {_time.time() - _t1:.1f}s")
    return np.concatenate([r["vals"].T.reshape(-1) for r in res.results])


def kernel(h_t, cache_h, cache_words):
    h_t, ch_sorted, cw_sorted = _prep(h_t, cache_h, cache_words)
    vals_sorted = run_device(h_t, ch_sorted, cw_sorted)
    return finish_on_host(vals_sorted, cw_sorted)


# revision 11
# speedup vs baseline: 2.3522x; 1.0413x over previous
"""Trainium2 Bass kernel for nn_Cache_28071906246843 (retrieval_knn).

reference semantics:
    q = h_t[cache_words]                         # [C, D] gather
    dist = sqrt(sum((cache_h - q)**2, -1))       # [C]
    vals = exp(dist / 32.0)                      # [C]
    cache_p = segment_sum(vals, cache_words, V)  # [V]
    out = log_softmax(cache_p[None, :])          # [1, V]

v5 design (all-pairs fp8 matmul, device returns only the cross term):
    dist^2_i = ||ch_i||^2 + ||w_{r(i)}||^2 - 2 ch_i . w_{r(i)}
Both norms are host-precomputed; the device computes ONLY the selected
-2*ch.w dot per element.  Cache elements are sorted by word id and split
into 8 shards of 16384; per supertile of 256 sorted elements the <=128
distinct h_t rows (scaled by -2, cast to fp8e4m3 on host along with ch,
both pre-transposed to contraction-major [128, 8, N] blocks) meet in 16
fp8 matmuls producing the all-pairs [256 elem, 128 word] dot in PSUM.
A host-built one-hot mask picks each element's own word: a tensor_tensor
multiply plus an X-axis reduce on DVE yield [128, 2] selected dots per
supertile.  No ScalarE activations, no indirect DMAs; sqrt/exp/
segment-sum/log_softmax run on the host.  The v4 dedup-gather kernel is
kept as a fallback in case a supertile exceeds 128 distinct words.
"""

import sys

import numpy as np

if "/opt/trn_rl_repo" not in sys.path:
    sys.path.insert(0, "/opt/trn_rl_repo")

import ml_dtypes

import concourse.bass as bass
import concourse.tile as tile
from concourse import bacc, mybir
from concourse.bass_utils import run_bass_kernel_spmd

V, D, C = 50257, 1024, 131072
NCORES = 8
CSH = C // NCORES  # 16384 elements per core
P = 128            # SBUF partitions
NT = CSH // P      # 128 element-tiles per core
SMOOTH = 32.0

SUP = 2            # element-tiles per supertile
NSUP = NT // SUP   # 64 supertiles per core
SUPW = SUP * P     # 256 elements per supertile
NCH = D // P       # 8 contraction chunks

FP8 = ml_dtypes.float8_e4m3


def build_nc_v5(ndmax, nsup: int = NSUP) -> bass.Bass:
    """All-pairs dot kernel.  Per supertile s:
      chb[s]: [128, 8, 256] fp8, chb[s][p][c][u] = ch[s*256+u, c*128+p]
      wb[s]:  [128, 8, 128] fp8, wb[s][p][c][j] = -2*ht[widx[s][j], c*128+p]
      ohb[s]: [128, 2, 128] fp8, ohb[s][m][g][w] = 1 iff elem g*128+m selects w
    PE: psum[m, g*128+w] = sum_c sum_p chb.T @ wb  (elements stationary, FWL)
    DVE: tmp = psum * ohb ; dsel[:, 2s+g] = reduce_X(tmp)
    """
    nc = bacc.Bacc(
        "TRN2", target_bir_lowering=False, debug=False, num_devices=NCORES
    )
    chb = nc.dram_tensor(
        "chb", [nsup, P, NCH * SUPW], mybir.dt.float8e4, kind="ExternalInput"
    )
    # wob = per-sup word block (8 chunks of -2*ht rows) ++ one-hot (2 tiles)
    wob = nc.dram_tensor(
        "wob", [nsup, P, (NCH + SUP) * P], mybir.dt.float8e4, kind="ExternalInput"
    )
    dsel = nc.dram_tensor(
        "dsel", [P, SUP * nsup], mybir.dt.float32, kind="ExternalOutput"
    )

    chb_ap = chb.ap()  # [nsup, 128, 2048]
    wob_ap = wob.ap()  # [nsup, 128, 1280]

    with tile.TileContext(nc) as tc:
        with (
            tc.tile_pool(name="io", bufs=8) as io,
            tc.tile_pool(name="tmpp", bufs=4) as tmpp,
            tc.tile_pool(name="psum", bufs=8, space="PSUM") as psum,
            tc.tile_pool(name="persist", bufs=1) as persist,
        ):
            dsel_sb = persist.tile([P, SUP * nsup], mybir.dt.float32)

            for s in range(nsup):
                n = int(ndmax[s])  # words actually used this supertile
                ch_sb = io.tile([P, NCH, SUPW], mybir.dt.float8e4, tag="ch")
                nc.sync.dma_start(out=ch_sb[:], in_=chb_ap[s])
                w_sb = io.tile([P, NCH + SUP, P], mybir.dt.float8e4, tag="w")
                nc.scalar.dma_start(out=w_sb[:], in_=wob_ap[s])

                pt = psum.tile([P, SUP, P], mybir.dt.float32, tag="pt")
                for g in range(SUP):
                    for c in range(NCH):
                        nc.tensor.matmul(
                            out=pt[:, g, 0:n],
                            lhsT=ch_sb[:, c, g * P : (g + 1) * P],
                            rhs=w_sb[:, c, 0:n],
                            start=(c == 0),
                            stop=(c == NCH - 1),
                        )

                tmp = tmpp.tile([P, SUP, P], mybir.dt.float32, tag="tmp")
                nc.vector.tensor_tensor(
                    out=tmp[:, :, 0:n],
                    in0=pt[:, :, 0:n],
                    in1=w_sb[:, NCH : NCH + SUP, 0:n],
                    op=mybir.AluOpType.mult,
                )
                nc.vector.tensor_reduce(
                    out=dsel_sb[:, SUP * s : SUP * (s + 1)],
                    in_=tmp[:, :, 0:n],
                    axis=mybir.AxisListType.X,
                    op=mybir.AluOpType.add,
                )

            nc.sync.dma_start(out=dsel.ap(), in_=dsel_sb[:])
    nc.compile()
    return nc


def prep_v5(h_t, ch_sorted, cw_sorted):
    """Host-side block building for v5.  Returns None if any supertile has
    more than 128 distinct words (fall back to v4 then)."""
    S = NCORES * NSUP  # 512 supertiles total
    seg = cw_sorted.reshape(S, SUPW)
    widx = np.empty((S, P), np.int64)
    rel = np.empty((S, SUPW), np.int64)
    nd = np.empty(S, np.int64)
    for s in range(S):
        uw, r = np.unique(seg[s], return_inverse=True)
        if len(uw) > P:
            return None
        nd[s] = len(uw)
        widx[s, : len(uw)] = uw
        widx[s, len(uw):] = uw[-1]
        rel[s] = r
    # SPMD: all cores share one program, so pad each supertile's word count
    # to the max across cores
    ndmax = nd.reshape(NCORES, NSUP).max(axis=0)

    ht8 = (-2.0 * h_t).astype(FP8)
    ch8 = ch_sorted.astype(FP8)

    # chb[s, p, c, u] = ch8[s*256+u, c*128+p]
    chb = np.ascontiguousarray(
        ch8.reshape(S, SUPW, NCH, P).transpose(0, 3, 2, 1)
    ).reshape(S, P, NCH * SUPW)
    # wb[s, p, c, j] = ht8[widx[s, j], c*128+p]
    wb = np.ascontiguousarray(
        ht8[widx].reshape(S, P, NCH, P).transpose(0, 3, 2, 1)
    ).reshape(S, P, NCH * P)
    # ohb[s, m, g, w] = 1 iff rel[s, g*128+m] == w
    ohb = np.zeros((S, P, SUP, P), FP8)
    s_i = np.repeat(np.arange(S), SUPW)
    u_i = np.tile(np.arange(SUPW), S)
    g_i, m_i = u_i // P, u_i % P
    ohb[s_i, m_i, g_i, rel.reshape(-1)] = 1.0
    ohb = ohb.reshape(S, P, SUP * P)

    wob = np.ascontiguousarray(
        np.concatenate([wb, ohb], axis=-1)
    )  # [S, P, 1280]

    hn2 = np.einsum("ij,ij->i", h_t, h_t, dtype=np.float64)
    cn2 = np.einsum("ij,ij->i", ch_sorted, ch_sorted, dtype=np.float64)
    b = cn2 + hn2[cw_sorted]  # [C] norm part of dist^2, in sorted order
    return chb, wob, b, ndmax


def make_in_maps_v5(chb, wob):
    in_maps = []
    for c in range(NCORES):
        sl = slice(c * NSUP, (c + 1) * NSUP)
        in_maps.append({"chb": chb[sl], "wob": wob[sl]})
    return in_maps


# ---------------------------------------------------------------------------
# v4 fallback (dedup gather + TensorE expand/subtract + ScalarE square)
# ---------------------------------------------------------------------------


def build_nc_v4(nt: int = NT, v: int = V, d: int = D) -> bass.Bass:
    nsup = nt // SUP
    nc = bacc.Bacc(
        "TRN2", target_bir_lowering=False, debug=False, num_devices=NCORES
    )
    ht = nc.dram_tensor("ht", [v, d], mybir.dt.float32, kind="ExternalInput")
    ch = nc.dram_tensor("ch", [nt * P, d], mybir.dt.float16, kind="ExternalInput")
    widx = nc.dram_tensor("widx", [P, nsup], mybir.dt.int32, kind="ExternalInput")
    nsel = nc.dram_tensor("nsel", [nt, P, P], mybir.dt.float16, kind="ExternalInput")
    ident = nc.dram_tensor("ident", [P, P], mybir.dt.float16, kind="ExternalInput")
    vals = nc.dram_tensor("vals", [P, nt], mybir.dt.float32, kind="ExternalOutput")

    ch_ap = ch.ap()
    nsel_ap = nsel.ap()

    with tile.TileContext(nc) as tc:
        with (
            tc.tile_pool(name="io", bufs=4) as io,
            tc.tile_pool(name="wpool", bufs=3) as wpool,
            tc.tile_pool(name="spool", bufs=4) as spool,
            tc.tile_pool(name="psum", bufs=4, space="PSUM") as psum,
            tc.tile_pool(name="scratch", bufs=2) as scratch,
            tc.tile_pool(name="persist", bufs=1) as persist,
        ):
            widx_sb = persist.tile([P, nsup], mybir.dt.int32)
            nc.sync.dma_start(out=widx_sb[:], in_=widx.ap())
            ident_sb = persist.tile([P, P], mybir.dt.float16)
            nc.sync.dma_start(out=ident_sb[:], in_=ident.ap())
            vals_sb = persist.tile([P, nt], mybir.dt.float32)
            d2_all = persist.tile([P, nt], mybir.dt.float32)

            for s in range(nsup):
                w_fp = wpool.tile([P, d], mybir.dt.float16, tag="wfp")
                nc.gpsimd.indirect_dma_start(
                    out=w_fp[:],
                    out_offset=None,
                    in_=ht.ap(),
                    in_offset=bass.IndirectOffsetOnAxis(
                        ap=widx_sb[:, s : s + 1], axis=0
                    ),
                )
                ch_sb = io.tile([P, SUP, d], mybir.dt.float16, tag="ch")
                ch_src = bass.AP(
                    tensor=ch_ap.tensor,
                    offset=s * SUPW * d,
                    ap=[[d, P], [P * d, SUP], [1, d]],
                )
                nc.sync.dma_start(out=ch_sb[:], in_=ch_src)
                ns_sb = spool.tile([P, SUP, P], mybir.dt.float16, tag="nsel")
                ns_src = bass.AP(
                    tensor=nsel_ap.tensor,
                    offset=s * SUP * P * P,
                    ap=[[P, P], [P * P, SUP], [1, P]],
                )
                nc.sync.dma_start(out=ns_sb[:], in_=ns_src)

                q_psums = []
                for k in range(SUP):
                    q_psum = psum.tile([P, d], mybir.dt.float32, tag="q")
                    q_psums.append(q_psum)
                    for h in range(0, d, 512):
                        nc.tensor.matmul(
                            out=q_psum[:, h : h + 512],
                            lhsT=ns_sb[:, k, :],
                            rhs=w_fp[:, h : h + 512],
                            start=True,
                            stop=(k == 1),
                        )
                for h in range(0, d, 512):
                    nc.tensor.matmul(
                        out=q_psums[0][:, h : h + 512],
                        lhsT=ident_sb[:],
                        rhs=ch_sb[:, 0, h : h + 512],
                        start=False,
                        stop=True,
                    )
                d_sb = io.tile([P, d], mybir.dt.float32, tag="dsb")
                nc.vector.tensor_tensor(
                    out=d_sb[:],
                    in0=ch_sb[:, 1, :],
                    in1=q_psums[1][:],
                    op=mybir.AluOpType.add,
                )
                t0 = SUP * s
                sq_tile = scratch.tile([P, d], mybir.dt.float32, tag="sq")
                nc.scalar.activation(
                    out=sq_tile[:],
                    in_=q_psums[0][:],
                    func=mybir.ActivationFunctionType.Square,
                    accum_out=d2_all[:, t0 : t0 + 1],
                )
                sq_tile2 = scratch.tile([P, d], mybir.dt.float32, tag="sq2")
                nc.scalar.activation(
                    out=sq_tile2[:],
                    in_=d_sb[:],
                    func=mybir.ActivationFunctionType.Square,
                    accum_out=d2_all[:, t0 + 1 : t0 + 2],
                )

            dist_all = persist.tile([P, nt], mybir.dt.float32)
            nc.scalar.activation(
                out=dist_all[:],
                in_=d2_all[:],
                func=mybir.ActivationFunctionType.Sqrt,
            )
            nc.scalar.activation(
                out=vals_sb[:],
                in_=dist_all[:],
                func=mybir.ActivationFunctionType.Exp,
                scale=1.0 / SMOOTH,
            )
            nc.sync.dma_start(out=vals.ap(), in_=vals_sb[:])
    nc.compile()
    return nc


def prep_v4(cw_sorted):
    widx_all, nsel_all = [], []
    neye = -np.eye(P, dtype=np.float16)
    for c in range(NCORES):
        shard = cw_sorted[c * CSH : (c + 1) * CSH]
        widx = np.empty((NSUP, P), np.int32)
        nsel = np.empty((NT, P, P), np.float16)
        for s in range(NSUP):
            seg = shard[s * SUPW : (s + 1) * SUPW]
            uw = np.unique(seg)
            if len(uw) > P:
                return None
            widx[s, : len(uw)] = uw
            widx[s, len(uw) :] = uw[-1]
            rel = np.searchsorted(uw, seg).reshape(SUP, P)
            for k in range(SUP):
                nsel[SUP * s + k] = neye[:, rel[k]]
        widx_all.append(np.ascontiguousarray(widx.T))
        nsel_all.append(nsel)
    return widx_all, nsel_all


def make_in_maps_v4(h_t, ch_sorted, widx_all, nsel_all):
    ident = np.eye(P, dtype=np.float16)
    in_maps = []
    for c in range(NCORES):
        sl = slice(c * CSH, (c + 1) * CSH)
        in_maps.append(
            {
                "ht": h_t,
                "ch": ch_sorted[sl].astype(np.float16),
                "widx": widx_all[c],
                "nsel": nsel_all[c],
                "ident": ident,
            }
        )
    return in_maps


def finish_on_host(vals_sorted, cw_sorted):
    """segment-sum + log_softmax (tiny O(C)+O(V) work)."""
    p = np.bincount(cw_sorted, weights=vals_sorted.astype(np.float64), minlength=V)
    m = p.max()
    lse = m + np.log(np.exp(p - m).sum())
    return (p - lse).astype(np.float32)[None, :]


def _prep(h_t, cache_h, cache_words):
    h_t = np.ascontiguousarray(np.asarray(h_t), dtype=np.float32)
    cache_h = np.ascontiguousarray(np.asarray(cache_h), dtype=np.float32)
    cw = np.asarray(cache_words).astype(np.int32)
    order = np.argsort(cw, kind="stable")
    return h_t, cache_h[order], cw[order]


def run_device(h_t, ch_sorted, cw_sorted, force_v1=False, verbose=False):
    """Compile + run the SPMD program; returns per-element vals (sorted order)."""
    import time as _time

    _t0 = _time.time()
    v5 = prep_v5(h_t, ch_sorted, cw_sorted)
    if v5 is not None:
        chb, wob, b, ndmax = v5
        nc = build_nc_v5(ndmax)
        in_maps = make_in_maps_v5(chb, wob)
        if verbose:
            print(f"[run_device] build+prep(v5): {_time.time() - _t0:.1f}s")
        _t1 = _time.time()
        res = run_bass_kernel_spmd(nc, in_maps, core_ids=list(range(NCORES)))
        if verbose:
            print(f"[run_device] compile+exec: {_time.time() - _t1:.1f}s")
        # dsel[p, t] = selected -2*dot for element t*128+p (per core)
        dsel = np.concatenate(
            [r["dsel"].T.reshape(-1) for r in res.results]
        ).astype(np.float64)
        d2 = np.maximum(b + dsel, 0.0)
        return np.exp(np.sqrt(d2) / SMOOTH)

    v4 = prep_v4(cw_sorted)
    assert v4 is not None, "both v5 and v4 prep failed"
    nc = build_nc_v4()
    in_maps = make_in_maps_v4(h_t, ch_sorted, *v4)
    if verbose:
        print(f"[run_device] build+prep(v4): {_time.time() - _t0:.1f}s")
    _t1 = _time.time()
    res = run_bass_kernel_spmd(nc, in_maps, core_ids=list(range(NCORES)))
    if verbose:
        print(f"[run_device] compile+exec: {_time.time() - _t1:.1f}s")
    return np.concatenate([r["vals"].T.reshape(-1) for r in res.results])


def kernel(h_t, cache_h, cache_words):
    h_t, ch_sorted, cw_sorted = _prep(h_t, cache_h, cache_words)
    vals_sorted = run_device(h_t, ch_sorted, cw_sorted)
    return finish_on_host(vals_sorted, cw_sorted)


# revision 17
# speedup vs baseline: 2.4754x; 1.0524x over previous
"""Trainium2 Bass kernel for nn_Cache_28071906246843 (retrieval_knn).

reference semantics:
    q = h_t[cache_words]                         # [C, D] gather
    dist = sqrt(sum((cache_h - q)**2, -1))       # [C]
    vals = exp(dist / 32.0)                      # [C]
    cache_p = segment_sum(vals, cache_words, V)  # [V]
    out = log_softmax(cache_p[None, :])          # [1, V]

v5 design (all-pairs fp8 matmul, device returns only the cross term):
    dist^2_i = ||ch_i||^2 + ||w_{r(i)}||^2 - 2 ch_i . w_{r(i)}
Both norms are host-precomputed; the device computes ONLY the selected
-2*ch.w dot per element.  Cache elements are sorted by word id and split
into 8 shards of 16384; per supertile of 256 sorted elements the <=128
distinct h_t rows (scaled by -2, cast to fp8e4m3 on host along with ch,
both pre-transposed to contraction-major [128, 8, N] blocks) meet in 16
fp8 matmuls producing the all-pairs [256 elem, 128 word] dot in PSUM.
A host-built one-hot mask picks each element's own word: a tensor_tensor
multiply plus an X-axis reduce on DVE yield [128, 2] selected dots per
supertile.  No ScalarE activations, no indirect DMAs; sqrt/exp/
segment-sum/log_softmax run on the host.  The v4 dedup-gather kernel is
kept as a fallback in case a supertile exceeds 128 distinct words.
"""

import sys

import numpy as np

if "/opt/trn_rl_repo" not in sys.path:
    sys.path.insert(0, "/opt/trn_rl_repo")

import ml_dtypes

import concourse.bass as bass
import concourse.tile as tile
from concourse import bacc, mybir
from concourse.bass_utils import run_bass_kernel_spmd

V, D, C = 50257, 1024, 131072
NCORES = 8
CSH = C // NCORES  # 16384 elements per core
P = 128            # SBUF partitions
NT = CSH // P      # 128 element-tiles per core
SMOOTH = 32.0

SUP = 2            # element-tiles per supertile
NSUP = NT // SUP   # 64 supertiles per core
SUPW = SUP * P     # 256 elements per supertile
NCH = D // P       # 8 contraction chunks
PAIR = 2           # supertiles per device loop iteration

FP8 = ml_dtypes.float8_e4m3


def build_nc_v5(ndmax, nsup: int = NSUP) -> bass.Bass:
    """All-pairs dot kernel.  Per supertile s:
      chb[s]: [128, 8, 256] fp8, chb[s][p][c][u] = ch[s*256+u, c*128+p]
      wb[s]:  [128, 8, 128] fp8, wb[s][p][c][j] = -2*ht[widx[s][j], c*128+p]
      ohb[s]: [128, 2, 128] fp8, ohb[s][m][g][w] = 1 iff elem g*128+m selects w
    PE: psum[m, g*128+w] = sum_c sum_p chb.T @ wb  (elements stationary, FWL)
    DVE: tmp = psum * ohb ; dsel[:, 2s+g] = reduce_X(tmp)
    """
    nc = bacc.Bacc(
        "TRN2", target_bir_lowering=False, debug=False, num_devices=NCORES
    )
    chb = nc.dram_tensor(
        "chb", [nsup, P, NCH * SUPW], mybir.dt.float8e4, kind="ExternalInput"
    )
    # wob = per-sup word block (8 chunks of -2*ht rows) ++ one-hot (2 tiles)
    wob = nc.dram_tensor(
        "wob", [nsup, P, (NCH + SUP) * P], mybir.dt.float8e4, kind="ExternalInput"
    )
    dsel = nc.dram_tensor(
        "dsel", [P, SUP * nsup], mybir.dt.float32, kind="ExternalOutput"
    )

    nq = nsup // PAIR
    chb_ap = chb.ap().rearrange(
        "(q h) p f -> q p h f", h=PAIR
    )  # [nq, 128, 2, 2048]
    wob_ap = wob.ap().rearrange("(q h) p f -> q p h f", h=PAIR)

    with tile.TileContext(nc) as tc:
        with (
            tc.tile_pool(name="io", bufs=6) as io,
            tc.tile_pool(name="tmpp", bufs=4) as tmpp,
            tc.tile_pool(name="psum", bufs=6, space="PSUM") as psum,
            tc.tile_pool(name="persist", bufs=1) as persist,
        ):
            dsel_sb = persist.tile([P, SUP * nsup], mybir.dt.float32)

            for q in range(nq):
                ch_sb = io.tile([P, PAIR, NCH, SUPW], mybir.dt.float8e4, tag="ch")
                nc.sync.dma_start(out=ch_sb[:], in_=chb_ap[q])
                w_sb = io.tile(
                    [P, PAIR, NCH + SUP, P], mybir.dt.float8e4, tag="w"
                )
                nc.scalar.dma_start(out=w_sb[:], in_=wob_ap[q])

                pt = psum.tile([P, PAIR, SUP, P], mybir.dt.float32, tag="pt")
                for h in range(PAIR):
                    # first use of each psum buf covers the full 128 cols so
                    # stale (possibly non-finite) PSUM data can't leak into
                    # the masked reduce below
                    n = P if q < 6 else int(ndmax[q * PAIR + h])
                    for g in range(SUP):
                        for c in range(NCH):
                            nc.tensor.matmul(
                                out=pt[:, h, g, 0:n],
                                lhsT=ch_sb[:, h, c, g * P : (g + 1) * P],
                                rhs=w_sb[:, h, c, 0:n],
                                start=(c == 0),
                                stop=(c == NCH - 1),
                            )

                tmp = tmpp.tile([P, PAIR, SUP, P], mybir.dt.float32, tag="tmp")
                nc.vector.tensor_tensor(
                    out=tmp[:],
                    in0=pt[:],
                    in1=w_sb[:, :, NCH : NCH + SUP, :],
                    op=mybir.AluOpType.mult,
                )
                nc.vector.tensor_reduce(
                    out=dsel_sb[:, SUP * PAIR * q : SUP * PAIR * (q + 1)],
                    in_=tmp[:],
                    axis=mybir.AxisListType.X,
                    op=mybir.AluOpType.add,
                )

            nc.sync.dma_start(out=dsel.ap(), in_=dsel_sb[:])
    nc.compile()
    return nc


def prep_v5(h_t, ch_sorted, cw_sorted):
    """Host-side block building for v5.  Returns None if any supertile has
    more than 128 distinct words (fall back to v4 then)."""
    S = NCORES * NSUP  # 512 supertiles total
    seg = cw_sorted.reshape(S, SUPW)
    widx = np.empty((S, P), np.int64)
    rel = np.empty((S, SUPW), np.int64)
    nd = np.empty(S, np.int64)
    for s in range(S):
        uw, r = np.unique(seg[s], return_inverse=True)
        if len(uw) > P:
            return None
        nd[s] = len(uw)
        widx[s, : len(uw)] = uw
        widx[s, len(uw):] = uw[-1]
        rel[s] = r
    # SPMD: all cores share one program, so pad each supertile's word count
    # to the max across cores
    ndmax = nd.reshape(NCORES, NSUP).max(axis=0)

    ht8 = (-2.0 * h_t).astype(FP8)
    ch8 = ch_sorted.astype(FP8)

    # chb[s, p, c, u] = ch8[s*256+u, c*128+p]
    chb = np.ascontiguousarray(
        ch8.reshape(S, SUPW, NCH, P).transpose(0, 3, 2, 1)
    ).reshape(S, P, NCH * SUPW)
    # wb[s, p, c, j] = ht8[widx[s, j], c*128+p]
    wb = np.ascontiguousarray(
        ht8[widx].reshape(S, P, NCH, P).transpose(0, 3, 2, 1)
    ).reshape(S, P, NCH * P)
    # ohb[s, m, g, w] = 1 iff rel[s, g*128+m] == w
    ohb = np.zeros((S, P, SUP, P), FP8)
    s_i = np.repeat(np.arange(S), SUPW)
    u_i = np.tile(np.arange(SUPW), S)
    g_i, m_i = u_i // P, u_i % P
    ohb[s_i, m_i, g_i, rel.reshape(-1)] = 1.0
    ohb = ohb.reshape(S, P, SUP * P)

    wob = np.ascontiguousarray(
        np.concatenate([wb, ohb], axis=-1)
    )  # [S, P, 1280]

    hn2 = np.einsum("ij,ij->i", h_t, h_t, dtype=np.float64)
    cn2 = np.einsum("ij,ij->i", ch_sorted, ch_sorted, dtype=np.float64)
    b = cn2 + hn2[cw_sorted]  # [C] norm part of dist^2, in sorted order
    return chb, wob, b, ndmax


def make_in_maps_v5(chb, wob):
    in_maps = []
    for c in range(NCORES):
        sl = slice(c * NSUP, (c + 1) * NSUP)
        in_maps.append({"chb": chb[sl], "wob": wob[sl]})
    return in_maps


# ---------------------------------------------------------------------------
# v4 fallback (dedup gather + TensorE expand/subtract + ScalarE square)
# ---------------------------------------------------------------------------


def build_nc_v4(nt: int = NT, v: int = V, d: int = D) -> bass.Bass:
    nsup = nt // SUP
    nc = bacc.Bacc(
        "TRN2", target_bir_lowering=False, debug=False, num_devices=NCORES
    )
    ht = nc.dram_tensor("ht", [v, d], mybir.dt.float32, kind="ExternalInput")
    ch = nc.dram_tensor("ch", [nt * P, d], mybir.dt.float16, kind="ExternalInput")
    widx = nc.dram_tensor("widx", [P, nsup], mybir.dt.int32, kind="ExternalInput")
    nsel = nc.dram_tensor("nsel", [nt, P, P], mybir.dt.float16, kind="ExternalInput")
    ident = nc.dram_tensor("ident", [P, P], mybir.dt.float16, kind="ExternalInput")
    vals = nc.dram_tensor("vals", [P, nt], mybir.dt.float32, kind="ExternalOutput")

    ch_ap = ch.ap()
    nsel_ap = nsel.ap()

    with tile.TileContext(nc) as tc:
        with (
            tc.tile_pool(name="io", bufs=4) as io,
            tc.tile_pool(name="wpool", bufs=3) as wpool,
            tc.tile_pool(name="spool", bufs=4) as spool,
            tc.tile_pool(name="psum", bufs=4, space="PSUM") as psum,
            tc.tile_pool(name="scratch", bufs=2) as scratch,
            tc.tile_pool(name="persist", bufs=1) as persist,
        ):
            widx_sb = persist.tile([P, nsup], mybir.dt.int32)
            nc.sync.dma_start(out=widx_sb[:], in_=widx.ap())
            ident_sb = persist.tile([P, P], mybir.dt.float16)
            nc.sync.dma_start(out=ident_sb[:], in_=ident.ap())
            vals_sb = persist.tile([P, nt], mybir.dt.float32)
            d2_all = persist.tile([P, nt], mybir.dt.float32)

            for s in range(nsup):
                w_fp = wpool.tile([P, d], mybir.dt.float16, tag="wfp")
                nc.gpsimd.indirect_dma_start(
                    out=w_fp[:],
                    out_offset=None,
                    in_=ht.ap(),
                    in_offset=bass.IndirectOffsetOnAxis(
                        ap=widx_sb[:, s : s + 1], axis=0
                    ),
                )
                ch_sb = io.tile([P, SUP, d], mybir.dt.float16, tag="ch")
                ch_src = bass.AP(
                    tensor=ch_ap.tensor,
                    offset=s * SUPW * d,
                    ap=[[d, P], [P * d, SUP], [1, d]],
                )
                nc.sync.dma_start(out=ch_sb[:], in_=ch_src)
                ns_sb = spool.tile([P, SUP, P], mybir.dt.float16, tag="nsel")
                ns_src = bass.AP(
                    tensor=nsel_ap.tensor,
                    offset=s * SUP * P * P,
                    ap=[[P, P], [P * P, SUP], [1, P]],
                )
                nc.sync.dma_start(out=ns_sb[:], in_=ns_src)

                q_psums = []
                for k in range(SUP):
                    q_psum = psum.tile([P, d], mybir.dt.float32, tag="q")
                    q_psums.append(q_psum)
                    for h in range(0, d, 512):
                        nc.tensor.matmul(
                            out=q_psum[:, h : h + 512],
                            lhsT=ns_sb[:, k, :],
                            rhs=w_fp[:, h : h + 512],
                            start=True,
                            stop=(k == 1),
                        )
                for h in range(0, d, 512):
                    nc.tensor.matmul(
                        out=q_psums[0][:, h : h + 512],
                        lhsT=ident_sb[:],
                        rhs=ch_sb[:, 0, h : h + 512],
                        start=False,
                        stop=True,
                    )
                d_sb = io.tile([P, d], mybir.dt.float32, tag="dsb")
                nc.vector.tensor_tensor(
                    out=d_sb[:],
                    in0=ch_sb[:, 1, :],
                    in1=q_psums[1][:],
                    op=mybir.AluOpType.add,
                )
                t0 = SUP * s
                sq_tile = scratch.tile([P, d], mybir.dt.float32, tag="sq")
                nc.scalar.activation(
                    out=sq_tile[:],
                    in_=q_psums[0][:],
                    func=mybir.ActivationFunctionType.Square,
                    accum_out=d2_all[:, t0 : t0 + 1],
                )
                sq_tile2 = scratch.tile([P, d], mybir.dt.float32, tag="sq2")
                nc.scalar.activation(
                    out=sq_tile2[:],
                    in_=d_sb[:],
                    func=mybir.ActivationFunctionType.Square,
                    accum_out=d2_all[:, t0 + 1 : t0 + 2],
                )

            dist_all = persist.tile([P, nt], mybir.dt.float32)
            nc.scalar.activation(
                out=dist_all[:],
                in_=d2_all[:],
                func=mybir.ActivationFunctionType.Sqrt,
            )
            nc.scalar.activation(
                out=vals_sb[:],
                in_=dist_all[:],
                func=mybir.ActivationFunctionType.Exp,
                scale=1.0 / SMOOTH,
            )
            nc.sync.dma_start(out=vals.ap(), in_=vals_sb[:])
    nc.compile()
    return nc


def prep_v4(cw_sorted):
    widx_all, nsel_all = [], []
    neye = -np.eye(P, dtype=np.float16)
    for c in range(NCORES):
        shard = cw_sorted[c * CSH : (c + 1) * CSH]
        widx = np.empty((NSUP, P), np.int32)
        nsel = np.empty((NT, P, P), np.float16)
        for s in range(NSUP):
            seg = shard[s * SUPW : (s + 1) * SUPW]
            uw = np.unique(seg)
            if len(uw) > P:
                return None
            widx[s, : len(uw)] = uw
            widx[s, len(uw) :] = uw[-1]
            rel = np.searchsorted(uw, seg).reshape(SUP, P)
            for k in range(SUP):
                nsel[SUP * s + k] = neye[:, rel[k]]
        widx_all.append(np.ascontiguousarray(widx.T))
        nsel_all.append(nsel)
    return widx_all, nsel_all


def make_in_maps_v4(h_t, ch_sorted, widx_all, nsel_all):
    ident = np.eye(P, dtype=np.float16)
    in_maps = []
    for c in range(NCORES):
        sl = slice(c * CSH, (c + 1) * CSH)
        in_maps.append(
            {
                "ht": h_t,
                "ch": ch_sorted[sl].astype(np.float16),
                "widx": widx_all[c],
                "nsel": nsel_all[c],
                "ident": ident,
            }
        )
    return in_maps


def finish_on_host(vals_sorted, cw_sorted):
    """segment-sum + log_softmax (tiny O(C)+O(V) work)."""
    p = np.bincount(cw_sorted, weights=vals_sorted.astype(np.float64), minlength=V)
    m = p.max()
    lse = m + np.log(np.exp(p - m).sum())
    return (p - lse).astype(np.float32)[None, :]


def _prep(h_t, cache_h, cache_words):
    h_t = np.ascontiguousarray(np.asarray(h_t), dtype=np.float32)
    cache_h = np.ascontiguousarray(np.asarray(cache_h), dtype=np.float32)
    cw = np.asarray(cache_words).astype(np.int32)
    order = np.argsort(cw, kind="stable")
    return h_t, cache_h[order], cw[order]


def run_device(h_t, ch_sorted, cw_sorted, force_v1=False, verbose=False):
    """Compile + run the SPMD program; returns per-element vals (sorted order)."""
    import time as _time

    _t0 = _time.time()
    v5 = prep_v5(h_t, ch_sorted, cw_sorted)
    if v5 is not None:
        chb, wob, b, ndmax = v5
        nc = build_nc_v5(ndmax)
        in_maps = make_in_maps_v5(chb, wob)
        if verbose:
            print(f"[run_device] build+prep(v5): {_time.time() - _t0:.1f}s")
        _t1 = _time.time()
        res = run_bass_kernel_spmd(nc, in_maps, core_ids=list(range(NCORES)))
        if verbose:
            print(f"[run_device] compile+exec: {_time.time() - _t1:.1f}s")
        # dsel[p, t] = selected -2*dot for element t*128+p (per core)
        dsel = np.concatenate(
            [r["dsel"].T.reshape(-1) for r in res.results]
        ).astype(np.float64)
        d2 = np.maximum(b + dsel, 0.0)
        return np.exp(np.sqrt(d2) / SMOOTH)

    v4 = prep_v4(cw_sorted)
    assert v4 is not None, "both v5 and v4 prep failed"
    nc = build_nc_v4()
    in_maps = make_in_maps_v4(h_t, ch_sorted, *v4)
    if verbose:
        print(f"[run_device] build+prep(v4): {_time.time() - _t0:.1f}s")
    _t1 = _time.time()
    res = run_bass_kernel_spmd(nc, in_maps, core_ids=list(range(NCORES)))
    if verbose:
        print(f"[run_device] compile+exec: {_time.time() - _t1:.1f}s")
    return np.concatenate([r["vals"].T.reshape(-1) for r in res.results])


def kernel(h_t, cache_h, cache_words):
    h_t, ch_sorted, cw_sorted = _prep(h_t, cache_h, cache_words)
    vals_sorted = run_device(h_t, ch_sorted, cw_sorted)
    return finish_on_host(vals_sorted, cw_sorted)
